# revision 1
# baseline (speedup 1.0000x reference)
"""Bass/Trainium2 kernel for nn_AttentionMemory (scatter_memory).

Reference computation (per batch b):
    S   = Mk^T @ Qk * (1/sqrt(CK))     # [HW, HW]
    P   = softmax(S, axis=memory)      # softmax over the m (row) axis
    out = mv @ P                       # [CV, HW]

Sharding: B=8 batches, one batch per NeuronCore (pure data parallel).

Per-core algorithm (HW=4096, CK=64, CV=512):
  - mk/qk cast to fp16 once (DVE), duplicated into both partition halves so
    S-matmuls (K=64) run pairwise-concurrent in the two PE row-halves.
  - mvT = mv^T via PE transposes (bf16), once.
  - For each q-group of 512 columns:
      * S[m,q] chunk tiles via fp16 matmuls (lhsT=mk chunk, rhs=qk group)
      * P = exp(S * scale) on ScalarE straight out of PSUM (bf16; bf16 is
        needed for range: S spans ~±205 here so exp reaches ~1e11).
        No max subtraction: softmax is shift-invariant and fp32/bf16 hold
        the range, so the result matches the reference.
      * Z[q] = colsum(P) via ones-vector matmuls, 4 chains packed into
        distinct PE column-groups (M=1 each) to run concurrently.
      * out_raw[c,q] = mvT^T @ P (bf16 matmul, fp32 accumulate)
      * out = out_raw * (1/Z) broadcast along partitions, DMA out.
"""

import numpy as np

import concourse.bass as bass
import concourse.mybir as mybir
import concourse.tile as tile
from concourse.masks import make_identity
from bass_rust import ScopedClock

B, CK, CV, H, W = 8, 64, 512, 64, 64
HW = H * W            # 4096
QG = 512              # q-group width (one PSUM bank of fp32)
NQ = HW // QG         # 8 q-groups
NM = HW // 128        # 32 m-chunks
NCB = CV // 128       # 4 c-blocks
SCALE = 1.0 / 8.0     # 1/sqrt(CK)

F32 = mybir.dt.float32
FP16 = mybir.dt.float16
BF16 = mybir.dt.bfloat16

PACK_S = True   # run S-matmul pairs in the two K=64 row-halves of the PE
PACK_Z = True   # run 4 Z-accumulation chains in distinct PE column-groups


class FixedTileContext(tile.TileContext):
    """Splits multi-wait sync_infos: this walrus accepts at most one sync
    wait per regular instruction (two on InstEventSemaphore). Extra waits
    move onto same-engine InstNoOp carriers inserted just before."""

    def _split_multi_waits(self, ordered):
        nc = self.nc
        for bb_name, insts in list(ordered.items()):
            new_insts = []
            changed = False
            for inst in insts:
                si = getattr(inst, "sync_info", None)
                waits = list(si.on_wait) if (si is not None and si.on_wait) else []
                limit = 2 if isinstance(inst, mybir.InstEventSemaphore) else 1
                if len(waits) > limit:
                    changed = True
                    for w in waits[limit:]:
                        new_insts.append(
                            mybir.InstNoOp(
                                name=nc.get_next_instruction_name(),
                                sync_info=mybir.SyncInfo(on_wait=[w], on_update=[]),
                                bass_nofuse=True,
                                engine=inst.engine,
                            )
                        )
                    inst.sync_info = mybir.SyncInfo(
                        on_wait=waits[:limit], on_update=list(si.on_update or [])
                    )
                new_insts.append(inst)
            if changed:
                ordered[bb_name] = new_insts

    def _lower_ordered_insts(self, ordered):
        self._split_multi_waits(ordered)
        return super()._lower_ordered_insts(ordered)

    def _drain_and_barrier(self, tick_clock, wait_clock):
        nc = self.nc
        drain_inst = nc.sync.drain()
        wait_clock.add_sem_waits(
            drain_inst.ins, ScopedClock({None: tick_clock.global_clock})
        )
        si = drain_inst.ins.sync_info
        waits = list(si.on_wait or []) if si is not None else []
        if len(waits) > 1:
            drain_inst.ins.sync_info = mybir.SyncInfo(
                on_wait=[waits[0]], on_update=list(si.on_update or [])
            )
            for w in waits[1:]:
                d2 = nc.sync.drain()
                d2.ins.sync_info = mybir.SyncInfo(on_wait=[w], on_update=[])
        nc.all_engine_barrier()
        assert self.sems is not None
        popped = nc._tile_sem_poison_stack.pop()
        assert popped is self._sem_poison
        nc.clear_and_free_semaphores(list(self.sems.allocated().values()))
        nc.all_engine_barrier()


def build_program(repeat: int = 1) -> bass.Bass:
    nc = bass.Bass()
    mk_d = nc.dram_tensor("Mk", [CK, HW], F32, kind="ExternalInput")
    qk_d = nc.dram_tensor("Qk", [CK, HW], F32, kind="ExternalInput")
    mv_d = nc.dram_tensor("mv", [CV, HW], F32, kind="ExternalInput")
    out_d = nc.dram_tensor("out", [CV, HW], F32, kind="ExternalOutput")

    with FixedTileContext(nc) as tc:
        with (
            tc.tile_pool(name="consts", bufs=1) as consts,
            tc.tile_pool(name="stage", bufs=2) as stage,
            tc.tile_pool(name="inp16", bufs=1) as inp16,
            tc.tile_pool(name="mvtp", bufs=1) as mvtp,
            tc.tile_pool(name="pp", bufs=2) as pp,
            tc.tile_pool(name="obp", bufs=3) as obp,
            tc.tile_pool(name="smallp", bufs=2) as smallp,
            tc.tile_pool(name="ps_s", bufs=2, space="PSUM") as ps_s_pool,
            tc.tile_pool(name="ps_o", bufs=4, space="PSUM") as ps_o_pool,
            tc.tile_pool(name="ps_z", bufs=1, space="PSUM") as ps_z_pool,
            tc.tile_pool(name="ps_r", bufs=1, space="PSUM") as ps_r_pool,
        ):
            identity = consts.tile([128, 128], F32)
            make_identity(nc, identity[:])
            ones_h = consts.tile([128, 1], BF16)
            nc.gpsimd.memset(ones_h[:], 1.0)
            ones_r = consts.tile([1, 128], F32)
            nc.gpsimd.memset(ones_r[:], 1.0)

            for _rep in range(repeat):
                # Load + cast mk, qk to fp16 (duplicated across partition halves
                # when PACK_S so the two PE row-halves can run concurrently).
                # repeat>1 re-runs the whole pipeline (for overhead-free timing).
                KP = 128 if PACK_S else CK
                mk16 = inp16.tile([KP, HW], FP16)
                qk16 = inp16.tile([KP, HW], FP16)
                st_mk = stage.tile([128, HW], F32, tag="stage")
                nc.sync.dma_start(st_mk[:CK, :], mk_d[:])
                nc.vector.tensor_copy(mk16[:CK, :], st_mk[:CK, :])
                st_qk = stage.tile([128, HW], F32, tag="stage")
                nc.sync.dma_start(st_qk[:CK, :], qk_d[:])
                nc.vector.tensor_copy(qk16[:CK, :], st_qk[:CK, :])
                if PACK_S:
                    nc.vector.tensor_copy(mk16[CK:, :], st_mk[:CK, :])
                    nc.vector.tensor_copy(qk16[CK:, :], st_qk[:CK, :])

                def s_phase(g):
                    """S matmuls + exp + Z accumulation for q-group g."""
                    qsl = slice(g * QG, (g + 1) * QG)
                    P = pp.tile([128, NM, QG], BF16, tag="P")
                    zw = 128 if PACK_Z else 1
                    ps_z = ps_z_pool.tile([zw, QG], F32, tag="z")
                    for j in range(NM):
                        half = (j % 2) if PACK_S else 0
                        ksl = slice(half * CK, half * CK + CK)
                        ps_sj = ps_s_pool.tile([128, QG], F32, tag="s")
                        nc.tensor.matmul(
                            ps_sj[:],
                            mk16[ksl, j * 128:(j + 1) * 128],
                            qk16[ksl, qsl],
                            start=True,
                            stop=True,
                        )
                        nc.scalar.activation(
                            P[:, j, :], ps_sj[:],
                            mybir.ActivationFunctionType.Exp, scale=SCALE,
                        )
                        # Z[q] += colsum of this chunk (keeps PE busy during exps)
                        if PACK_Z:
                            col = j % 4
                            nc.tensor.matmul(
                                ps_z[32 * col:32 * col + 1, :], ones_h[:],
                                P[:, j, :],
                                start=(j < 4), stop=(j >= NM - 4),
                                tile_position=(0, 32 * col),
                            )
                        else:
                            nc.tensor.matmul(
                                ps_z[:], ones_h[:], P[:, j, :],
                                start=(j == 0), stop=(j == NM - 1),
                            )
                    return qsl, P, ps_z

                # mvT[p, j, c] = mv[c, j*128+p], bf16
                mvT = mvtp.tile([128, NM, CV], BF16)
                for cb in range(NCB):
                    mv_sb = stage.tile([128, HW], F32, tag="stage")
                    nc.sync.dma_start(mv_sb[:], mv_d[cb * 128:(cb + 1) * 128, :])
                    for j in range(NM):
                        ps_t = ps_o_pool.tile([128, 128], F32, tag="o")
                        nc.tensor.transpose(
                            ps_t[:], mv_sb[:, j * 128:(j + 1) * 128], identity[:]
                        )
                        nc.vector.tensor_copy(
                            mvT[:, j, cb * 128:(cb + 1) * 128], ps_t[:]
                        )

                for g in range(NQ):
                    qsl, P, ps_z = s_phase(g)

                    rz = smallp.tile([1, QG], F32, tag="rz")
                    if PACK_Z:
                        # combine the 4 column-group partials; VectorE may read
                        # at most one PSUM operand per instruction, so chain
                        # through SBUF.
                        za = smallp.tile([1, QG], F32, tag="za")
                        nc.vector.tensor_copy(za[:], ps_z[0:1, :])
                        zb = smallp.tile([1, QG], F32, tag="zb")
                        nc.vector.tensor_tensor(
                            out=zb[:], in0=za[:], in1=ps_z[32:33, :],
                            op=mybir.AluOpType.add,
                        )
                        zc = smallp.tile([1, QG], F32, tag="zc")
                        nc.vector.tensor_tensor(
                            out=zc[:], in0=zb[:], in1=ps_z[64:65, :],
                            op=mybir.AluOpType.add,
                        )
                        zs = smallp.tile([1, QG], F32, tag="zs")
                        nc.vector.tensor_tensor(
                            out=zs[:], in0=zc[:], in1=ps_z[96:97, :],
                            op=mybir.AluOpType.add,
                        )
                        nc.vector.reciprocal(rz[:], zs[:])
                    else:
                        nc.vector.reciprocal(rz[:], ps_z[:])

                    # broadcast rz along partitions: ones[1,128]^T @ rz[1,QG]
                    ps_rzb = ps_r_pool.tile([128, QG], F32, tag="rzb")
                    nc.tensor.matmul(
                        ps_rzb[:], ones_r[:], rz[:], start=True, stop=True,
                    )
                    rzb = smallp.tile([128, QG], F32, tag="rzb_sb")
                    nc.vector.tensor_copy(rzb[:], ps_rzb[:])

                    for cb in range(NCB):
                        ps_o = ps_o_pool.tile([128, QG], F32, tag="o")
                        for j in range(NM):
                            nc.tensor.matmul(
                                ps_o[:],
                                mvT[:, j, cb * 128:(cb + 1) * 128],
                                P[:, j, :],
                                start=(j == 0),
                                stop=(j == NM - 1),
                            )
                        o_sb = obp.tile([128, QG], F32, tag="ob")
                        nc.vector.tensor_tensor(
                            out=o_sb[:], in0=ps_o[:], in1=rzb[:],
                            op=mybir.AluOpType.mult,
                        )
                        nc.sync.dma_start(
                            out_d[cb * 128:(cb + 1) * 128, qsl], o_sb[:]
                        )
    return nc


_prog_cache = None


def _get_program():
    global _prog_cache
    if _prog_cache is None:
        _prog_cache = build_program()
    return _prog_cache


def run(inputs, **spmd_kwargs):
    from concourse.bass_utils import run_bass_kernel_spmd

    Mk = np.ascontiguousarray(np.asarray(inputs["Mk"], dtype=np.float32))
    Qk = np.ascontiguousarray(np.asarray(inputs["Qk"], dtype=np.float32))
    mv = np.ascontiguousarray(np.asarray(inputs["mv"], dtype=np.float32))
    assert Mk.shape == (B, CK, H, W) and Qk.shape == (B, CK, H, W)
    assert mv.shape == (B, CV, H, W)

    in_maps = [
        {
            "Mk": Mk[b].reshape(CK, HW),
            "Qk": Qk[b].reshape(CK, HW),
            "mv": mv[b].reshape(CV, HW),
        }
        for b in range(B)
    ]
    nc = _get_program()
    res = run_bass_kernel_spmd(nc, in_maps, list(range(B)), **spmd_kwargs)
    out = np.stack([res.results[b]["out"] for b in range(B)])
    return out.reshape(B, CV, H, W).astype(np.float32), res


def kernel(**inputs) -> np.ndarray:
    out, _ = run(inputs)
    return out

